# revision 1
# baseline (speedup 1.0000x reference)
"""Multi-head attention (B=2,T=2048,D=1024,H=16,DK=64, causal, RoPE) on 8 TRN2 cores.

Sharding: data-parallel over batch (2) x tensor-parallel over heads (16 -> 4 per
core). core = 4*b + g handles batch b, heads [4g..4g+3]. RoPE tables replicated.
Host pre-transposes x and the projection weights, and permutes the q/k head dims
into [x1(32); x2(32)] blocks per head so RoPE is pure elementwise work on chip.
Each core returns a partial output projection; the host sums the 4 head-group
partials per batch and adds the output bias.
"""

import sys

for _p in ("/opt/trn_rl_repo", "/root/.axon_site/_ro/trn_rl_repo"):
    if _p not in sys.path:
        sys.path.append(_p)

import numpy as np

from concourse import bacc, tile, mybir
import concourse.bass as bass
from concourse.bass2jax import _bass_exec_p, install_neuronx_cc_hook

B, T, D, H, DK = 2, 2048, 1024, 16, 64
G = 4          # heads per core
DSH = G * DK   # 256 sharded head dims per core
NCORES = 8
KT = D // 128  # 8 contraction tiles for projections
NTT = T // 128  # 16 row tiles
NCH = T // 512  # 4 column chunks
F32 = mybir.dt.float32
F32R = mybir.dt.float32r
BF16 = mybir.dt.bfloat16

_CACHE = {}


DEBUG_DUMPS = False
PHASES = frozenset({1, 2, 3})
DIAG_MASKS = True


def _build_bass():
    nc = bacc.Bacc("TRN2", target_bir_lowering=False, debug=False)

    xT = nc.dram_tensor("xT", [D, T], BF16, kind="ExternalInput").ap()
    wqT = nc.dram_tensor("wqT", [128, KT * DSH], BF16, kind="ExternalInput").ap()
    wkT = nc.dram_tensor("wkT", [128, KT * DSH], BF16, kind="ExternalInput").ap()
    wvT = nc.dram_tensor("wvT", [128, KT * DSH], BF16, kind="ExternalInput").ap()
    woT = nc.dram_tensor("woT", [128, 2 * D], BF16, kind="ExternalInput").ap()
    bqk = nc.dram_tensor("bqk", [128, 4], F32, kind="ExternalInput").ap()
    bv = nc.dram_tensor("bv", [1, DSH], F32, kind="ExternalInput").ap()
    cc = nc.dram_tensor("cc", [128, T], BF16, kind="ExternalInput").ap()
    ss = nc.dram_tensor("ss", [128, T], BF16, kind="ExternalInput").ap()
    m01 = nc.dram_tensor("m01", [128, 128], BF16, kind="ExternalInput").ap()
    ones = nc.dram_tensor("ones", [1, 128], F32, kind="ExternalInput").ap()
    out = nc.dram_tensor("out", [T, D], BF16, kind="ExternalOutput").ap()

    def r(ap):  # fp32 storage -> fp32r matmul operand
        return ap.bitcast(F32R)

    with tile.TileContext(nc) as tc:
        with (
            tc.tile_pool(name="const", bufs=1) as const,
            tc.tile_pool(name="persist", bufs=1) as persist,
            tc.tile_pool(name="rope", bufs=2) as ropep,
            tc.tile_pool(name="attn", bufs=2) as attnp,
            tc.tile_pool(name="epi", bufs=2) as epip,
        ):
            # ---- resident tensors; DMAs issued in consumption order ----
            # sync + scalar are the two HWDGE queues (low first-byte
            # latency); gpsimd DMAs ride the software DGE.  HBM bandwidth is
            # one shared ~360GB/s pool, so the projection weights stream
            # per-k-tile in the exact order chunk 0 consumes them (sync
            # queue) while xt tiles stream on scalar; big late-use constants
            # (cc/ss/m01/wo) are issued later, from inside the chunk loop.
            wq_sb = const.tile([128, KT, DSH], BF16)
            wk_sb = const.tile([128, KT, DSH], BF16)
            wv_sb = const.tile([128, KT, DSH], BF16)
            hw = KT // 2 * DSH
            for w_sb, w_dram, eng in ((wq_sb, wqT, nc.sync), (wk_sb, wkT, nc.gpsimd),
                                      (wv_sb, wvT, nc.gpsimd)):
                wf = w_sb.rearrange("p k n -> p (k n)")
                eng.dma_start(out=wf[:, 0:hw], in_=w_dram[:, 0:hw])
            # the whole of x stays resident: 8 k-row-blocks of [128, T] with
            # 4KB contiguous per partition line (DMA descriptor count is the
            # binding constraint, not bandwidth).  Alternating queues; phase
            # 1 then never touches HBM for x again.
            xk = [const.tile([128, T], BF16, name=f"xk{_k}") for _k in range(KT)]
            for k in (0, 1):
                eng = nc.sync if k % 2 == 0 else nc.scalar
                eng.dma_start(out=xk[k], in_=xT[128 * k : 128 * k + 128, :])
            # tiny consts: each pays ~1us of fixed DMA latency, so they ride
            # behind the first x blocks, well before their ~20us first use
            bqk_sb = const.tile([128, 4], F32)
            nc.sync.dma_start(out=bqk_sb, in_=bqk)
            bv_sb = const.tile([1, DSH], F32R)
            nc.sync.dma_start(out=bv_sb, in_=bv.bitcast(F32R))
            m01_sb = const.tile([128, 128], BF16)
            nc.scalar.dma_start(out=m01_sb, in_=m01)
            ones_sb = const.tile([1, 128], F32R)
            nc.vector.memset(ones_sb.bitcast(F32), 1.0)
            for k in range(2, KT):
                eng = nc.sync if k % 2 == 0 else nc.scalar
                eng.dma_start(out=xk[k], in_=xT[128 * k : 128 * k + 128, :])
            cc_sb = const.tile([128, T], BF16)
            ss_sb = const.tile([128, T], BF16)
            nc.scalar.dma_start(out=cc_sb, in_=cc)
            nc.scalar.dma_start(out=ss_sb, in_=ss)
            wo_sb = const.tile([128, 2, D], BF16)

            qT_sb = persist.tile([128, 2, T], BF16)   # [d-tile, t], heads 2*dt+{0,1}
            kT_sb = persist.tile([128, 2, T], BF16)
            v1_sb = persist.tile([128, G, NTT, 65], BF16)  # [s, head, s-tile, d|1]
            # only the ones-column needs init (softmax denominators); cols
            # 0:64 are fully overwritten by the v evacuations
            nc.vector.memset(v1_sb[:, :, :, 64:65], 1.0)
            ctxT_sb = persist.tile([128, 2, T], BF16)

            # ---- phase 1: projections + RoPE, one 512-wide t-chunk at a time ----
            if 1 not in PHASES:
                raise RuntimeError
            with tc.tile_pool(name="ps1", bufs=1, space="PSUM") as ps1:
                # PE warm-up: ~5us of tiny matmuls on a zeroed scratch tile
                # (no DMA dependency at all) while the input DMAs land, so
                # the HAM clock-gate releases to 2.4GHz before the first real
                # projection matmul.  The zeros land in the qp0 slot and are
                # overwritten by chunk 0's start=True accumulation.
                wj = const.tile([128, 128], BF16)
                nc.vector.memset(wj, 0.0)
                wt = ps1.tile([128, 512], F32, tag="qp0", name="warm")
                for _ in range(56):  # K=128 so the HAM sees a busy array
                    nc.tensor.matmul(wt[:, 0:128], wj, wj, start=True, stop=True)
                for tch in range(NCH):
                    tsl = slice(512 * tch, 512 * tch + 512)
                    if tch == 2:
                        nc.gpsimd.dma_start(
                            out=wo_sb.rearrange("p k n -> p (k n)"), in_=woT)
                    qp = [ps1.tile([128, 512], F32, tag=f"qp{dt}", name=f"qp{dt}") for dt in range(2)]
                    kp = [ps1.tile([128, 512], F32, tag=f"kp{dt}", name=f"kp{dt}") for dt in range(2)]
                    # NOTE: one accumulation chain per PSUM bank -- packing
                    # two v chains into one bank corrupts the accumulation
                    vp4 = [ps1.tile([128, 256], F32, tag=f"vp{i}", name=f"vp{i}")
                           for i in range(4)]

                    class _VP:  # keep the [tt//2][:, tt%2, :] indexing
                        def __init__(self, a, b): self.t = (a, b)
                        def __getitem__(self, key): return self.t[key[1]]
                    vp = [_VP(vp4[0], vp4[1]), _VP(vp4[2], vp4[3])]
                    if tch == 0:
                        # back k-halves of the projection weights ride the
                        # software-DGE queue (idle until the first RoPE swap)
                        for w_sb, w_dram in ((wq_sb, wqT), (wk_sb, wkT), (wv_sb, wvT)):
                            wf = w_sb.rearrange("p k n -> p (k n)")
                            nc.gpsimd.dma_start(out=wf[:, hw:], in_=w_dram[:, hw:])
                    for k in range(KT):
                        xt = xk[k][:, tsl]
                        # v first: vp is double-buffered, so at a chunk
                        # boundary the PE can start these while qp/kp of the
                        # previous chunk are still being evacuated
                        for tt in range(4):
                            nc.tensor.matmul(
                                vp[tt // 2][:, tt % 2, :],
                                xt[:, 128 * tt : 128 * tt + 128],
                                wv_sb[:, k, :],
                                start=(k == 0), stop=False)
                        for dt in range(2):
                            dsl = slice(128 * dt, 128 * dt + 128)
                            nc.tensor.matmul(qp[dt], wq_sb[:, k, dsl], xt,
                                             start=(k == 0), stop=(k == KT - 1))
                            nc.tensor.matmul(kp[dt], wk_sb[:, k, dsl], xt,
                                             start=(k == 0), stop=(k == KT - 1))
                    for tt in range(4):  # + bv broadcast along t (rank-1 matmul)
                        nc.tensor.matmul(vp[tt // 2][:, tt % 2, :], ones_sb, bv_sb,
                                         start=False, stop=True)
                    def rope_qk():
                        # q/k bias add + RoPE (frees qp/kp so the next
                        # chunk's projections restart immediately)
                        for dt in range(2):
                            for which, psumt, dst in ((0, qp[dt], qT_sb), (1, kp[dt], kT_sb)):
                                raw = ropep.tile([128, 512], BF16,
                                                 tag=f"raw{which}{dt}", bufs=2)
                                if which == 0:
                                    nc.scalar.activation(
                                        out=raw, in_=psumt,
                                        func=mybir.ActivationFunctionType.Identity,
                                        bias=bqk_sb[:, 2 * which + dt : 2 * which + dt + 1])
                                else:
                                    nc.vector.tensor_scalar_add(
                                        raw, psumt,
                                        bqk_sb[:, 2 * which + dt : 2 * which + dt + 1])
                                swp = ropep.tile([128, 512], BF16, tag="swp", bufs=2)
                                for blk in range(4):
                                    # SWDGE: off the HWDGE queues so the xt
                                    # stream never stalls at a chunk boundary
                                    sb = blk ^ 1
                                    nc.gpsimd.dma_start(
                                        out=swp[32 * blk : 32 * blk + 32, :],
                                        in_=raw[32 * sb : 32 * sb + 32, :])
                                t1 = ropep.tile([128, 512], BF16, tag="t1", bufs=2)
                                t2 = ropep.tile([128, 512], BF16, tag="t2", bufs=2)
                                nc.vector.tensor_mul(t1, raw, cc_sb[:, tsl])
                                nc.vector.tensor_mul(t2, swp, ss_sb[:, tsl])
                                nc.vector.tensor_add(dst[:, dt, tsl], t1, t2)

                    def v_copies():
                        # v -> [s, d] bf16 slices per head (ones col untouched)
                        for tt in range(4):
                            st = 4 * tch + tt
                            nc.vector.tensor_copy(v1_sb[:, :, st, 0:64],
                                                  vp[tt // 2][:, tt % 2, :])
                    if tch < NCH - 1:
                        rope_qk(); v_copies()
                    else:
                        # last chunk: the v copies gate the release of the
                        # PSUM banks that phase 2's score tiles reuse -- do
                        # them first; chunk 3's RoPE overlaps chunk 2's scores
                        v_copies(); rope_qk()

            # ---- phase 2+3: causal attention (transposed scores) with the
            # output projection interleaved per 512-wide q-chunk ----
            # Head pairs (2*dt, 2*dt+1): the odd head's q/k rows live at
            # partition 64, so its score matmuls land in PE row-groups 2-3 and
            # run concurrently with the even head's.
            with tc.tile_pool(name="ps2", bufs=1, space="PSUM") as ps2:
                # Fine-grained software pipeline.  The score stream of each
                # (chunk, head-pair) position is ACT(exp)-paced: the PE only
                # fills ~40% of it.  The previous position's PV / epilogue /
                # out-projection matmuls are emitted as "filler" closures
                # between score pairs so the PE and ACT both stay busy (and
                # the PE never idles into a HAM re-throttle).  Consecutive
                # positions alternate head-pair parity, so the at-tile tags
                # of the score stream and the PV fillers never collide.
                def emit_scores(j, dt, fillers):
                    qsl = slice(512 * j, 512 * j + 512)
                    nst = 4 * j + 4  # s-tiles needed (incl. diagonal)
                    ats = [attnp.tile([128, NTT, 512], BF16, tag=f"at{dt}{i}",
                                      name=f"at{dt}{i}", bufs=1) for i in range(2)]
                    scs = [ps2.tile([128, 1024], F32, tag=f"sc{i}",
                                    name=f"sc{i}", bufs=1) for i in range(2)]
                    for p2 in range(nst // 2):  # scores + exp, 2 s-tiles a time
                        # the last pair holds diagonal s-tiles 4j+2/4j+3
                        # whose q-columns < 256 are fully masked: skip them
                        co = 256 if p2 == nst // 2 - 1 else 0
                        for i in range(2):
                            st = 2 * p2 + i
                            for hh in range(2):  # adjacent mms pack rows 0-63/64-127
                                rsl = slice(64 * hh, 64 * hh + 64)
                                nc.tensor.matmul(
                                    scs[hh][:, 512 * i + co : 512 * i + 512],
                                    kT_sb[rsl, dt, 128 * st : 128 * st + 128],
                                    qT_sb[rsl, dt, 512 * j + co : 512 * j + 512],
                                    start=True, stop=True)
                        for hh in range(2):
                            sc_v = scs[hh].rearrange("p (i c) -> p i c", i=2)[:, :, co:]
                            nc.scalar.activation(
                                out=ats[hh][:, 2 * p2 : 2 * p2 + 2, co:], in_=sc_v,
                                func=mybir.ActivationFunctionType.Exp, scale=0.125)
                        # each pair feeds ~2.2us of exp (two 1024-col
                        # activations) while its own score matmuls take
                        # ~0.9us: pop ~1.3us of PE filler to match (more on
                        # the tiny final chunk, to drain the backlog)
                        budget = 1.3 if j != 0 else 3.0
                        while fillers and budget > 0:
                            cost, f = fillers.pop(0)
                            f()
                            budget -= cost
                    return ats, qsl, nst

                def make_fillers(j, dt, ats, qsl, nst):
                    """PV + softmax epilogue of position (j, dt), as
                    (PE-cost-us, closure) pairs."""
                    fl = []
                    for hh in (1, 0):  # hh=1 first: its ctxT write goes via
                        # a DMA whose ~2us completion latency then hides
                        # under the hh=0 PV chain
                        h = 2 * dt + hh
                        at = ats[hh]

                        def diag(at=at, j=j):
                            # causal fixup: the 4 diagonal blocks sit at free
                            # offsets (4j+c)*512 + 128c (stride 640); mask all
                            # four with one strided multiply by m01
                            base = at[:, 4 * j, 0:128]
                            diag_ap = bass.AP(
                                tensor=base.tensor, offset=base.offset,
                                ap=[list(base.ap[0]), [640, 4], [1, 128]])
                            m01_b = bass.AP(
                                tensor=m01_sb.tensor, offset=m01_sb.offset,
                                ap=[list(m01_sb.ap[0]), [0, 4], [1, 128]])
                            nc.vector.tensor_mul(diag_ap, diag_ap, m01_b)
                        fl.append((0.0, diag))
                        ct = ps2.tile([65, 512], F32, tag=f"ct{hh}", name=f"ct{hh}", bufs=1)
                        for st0 in range(0, nst, 2):
                            def pv(ct=ct, at=at, h=h, st0=st0, j=j, nst=nst):
                                for st in (st0, st0 + 1):
                                    # diagonal s-tiles: columns < 128c are
                                    # fully masked -- exclude them from the
                                    # matmul instead of zeroing attnT
                                    c = max(st - 4 * j, 0)
                                    nc.tensor.matmul(
                                        ct[:, 128 * c :], v1_sb[:, h, st, :],
                                        at[:, st, 128 * c :],
                                        start=(st == 0), stop=(st == nst - 1))
                            fl.append((0.43, pv))

                        def epi(ct=ct, hh=hh, dt=dt, qsl=qsl):
                            rr = epip.tile([1, 512], F32, tag="rr")
                            RECIP_APPROX = True
                            if RECIP_APPROX:
                                # custom-DVE ops read SBUF only: stage the
                                # PSUM denominator row first
                                dn = epip.tile([1, 512], F32, tag="dn")
                                nc.vector.tensor_copy(dn, ct[64:65, :])
                                nc.vector.reciprocal_approx_fast(out=rr, in_=dn)
                            else:
                                nc.vector.reciprocal(rr, ct[64:65, :])
                            rb = epip.tile([64, 512], F32, tag="rb")
                            nc.gpsimd.partition_broadcast(rb, rr)
                            if hh == 0:
                                nc.vector.tensor_mul(ctxT_sb[0:64, dt, qsl], ct[0:64, :], rb)
                            else:
                                stg = epip.tile([64, 512], BF16, tag="stg")
                                nc.vector.tensor_mul(stg, ct[0:64, :], rb)
                                nc.sync.dma_start(out=ctxT_sb[64:128, dt, qsl], in_=stg)
                        fl.append((0.05, epi))
                    return fl

                def po_fillers(j):
                    """Out-projection of chunk j (4 t-tiles x 2 psum-bank
                    halves, double-buffered), as (PE-cost, closure) pairs."""
                    fl = []
                    for tt in range(4 * j, 4 * j + 4):
                        for nchk in range(2):
                            po = ps2.tile([128, 512], F32, tag="po", name="po",
                                          bufs=2)

                            def pomm(po=po, tt=tt, nchk=nchk):
                                for k in range(2):
                                    nc.tensor.matmul(
                                        po,
                                        ctxT_sb[:, k, 128 * tt : 128 * tt + 128],
                                        wo_sb[:, k, 512 * nchk : 512 * nchk + 512],
                                        start=(k == 0), stop=(k == 1))
                            fl.append((0.43, pomm))

                            def poev(po=po, tt=tt, nchk=nchk):
                                osb = epip.tile([128, 512], BF16, tag="osb", bufs=3)
                                nc.vector.tensor_copy(osb, po)
                                nc.sync.dma_start(
                                    out=out[128 * tt : 128 * tt + 128,
                                            512 * nchk : 512 * nchk + 512],
                                    in_=osb)
                            fl.append((0.1, poev))
                    return fl

                # bridge the phase transition (while the last RoPE chunk
                # finishes on the DVE) with full-array junk matmuls so the
                # HAM never re-throttles, and seed the early score-only
                # positions with a few more as fillers
                wjt = ps2.tile([128, 512], F32, tag="po", name="warm2", bufs=2)
                for _ in range(24):
                    nc.tensor.matmul(wjt[:, 0:128], wj, wj, start=True, stop=True)

                def junk(_=None):
                    nc.tensor.matmul(wjt[:, 0:128], wj, wj, start=True, stop=True)

                # chunk 2 first: its kT/qT chunks finished RoPE while chunk
                # 3 was still projecting, so the scores can start with zero
                # transition stall (chunk 3's RoPE hides under them); the
                # tiny chunk 0 last keeps the kernel tail short.
                seq = [(2, 0), (2, 1), (3, 0), (3, 1), (1, 0), (1, 1), (0, 0), (0, 1)]
                fillers = [(0.3, junk)] * 16
                prev_chunk_done = None
                for j, dt in seq:
                    if dt == 0 and prev_chunk_done is not None:
                        fillers += po_fillers(prev_chunk_done)
                    ats, qsl, nst = emit_scores(j, dt, fillers)
                    fillers += make_fillers(j, dt, ats, qsl, nst)
                    if dt == 1:
                        prev_chunk_done = j
                for _, f in fillers:  # flush: leftovers + PV/epilogue of (0,1)
                    f()
                for _, f in po_fillers(0):
                    f()

    nc.compile()
    return nc


def _make_tables():
    i = np.arange(0, DK, 2, dtype=np.float32) / DK  # 2i/DK
    theta = 10000.0 ** i  # [32]
    pos = np.arange(T, dtype=np.float32)
    ang = pos[None, :] / theta[:, None]  # [32, T]
    sinT, cosT = np.sin(ang), np.cos(ang)
    import ml_dtypes
    cc = np.tile(cosT, (4, 1)).astype(ml_dtypes.bfloat16)  # [128, T]
    ss = np.tile(np.concatenate([-sinT, sinT], 0), (2, 1)).astype(ml_dtypes.bfloat16)
    m01 = (np.arange(128)[:, None] <= np.arange(128)[None, :]).astype(ml_dtypes.bfloat16)
    return cc, ss, m01


def _make_in_maps(x, wq, bq, wk, bk, wv, bv, wo):
    cc, ss, m01 = _make_tables()
    p = np.concatenate([np.arange(0, DK, 2), np.arange(1, DK, 2)])  # rope perm
    in_maps = []
    for core in range(NCORES):
        b, g = divmod(core, G)
        heads = np.arange(4 * g, 4 * g + 4)
        rows_qk = np.concatenate([64 * h + p for h in heads])
        rows_v = np.concatenate([64 * h + np.arange(DK) for h in heads])
        bqk = np.stack([bq[rows_qk[0:128]], bq[rows_qk[128:256]],
                        bk[rows_qk[0:128]], bk[rows_qk[128:256]]], axis=1)
        import ml_dtypes
        bf = ml_dtypes.bfloat16
        def wtile(w):  # [D, DSH] -> [128, KT*DSH] matching sbuf [p, k, n]
            return np.ascontiguousarray(
                w.reshape(KT, 128, DSH).transpose(1, 0, 2).reshape(128, KT * DSH))
        woTl = wo[:, rows_v].T.astype(bf)  # [DSH, D]
        woTl = woTl.reshape(2, 128, D).transpose(1, 0, 2).reshape(128, 2 * D)
        in_maps.append({
            "xT": np.ascontiguousarray(x[b].T.astype(bf)),
            "wqT": wtile(wq[rows_qk].T.astype(bf)),
            "wkT": wtile(wk[rows_qk].T.astype(bf)),
            "wvT": wtile(wv[rows_v].T.astype(bf)),
            "woT": np.ascontiguousarray(woTl),
            "bqk": np.ascontiguousarray(bqk.astype(np.float32)),
            "bv": np.ascontiguousarray(bv[rows_v][None, :]),
            "cc": cc, "ss": ss, "m01": m01,
            "ones": np.ones((1, 128), np.float32),
        })
    return in_maps


def _get_runner():
    """Compile once; return a jitted 8-core runner reusable across calls."""
    if "runner" in _CACHE:
        return _CACHE["runner"]
    import jax
    from jax.sharding import Mesh, PartitionSpec
    from jax.experimental.shard_map import shard_map

    install_neuronx_cc_hook()
    nc = _build_bass()

    partition_name = nc.partition_id_tensor.name if nc.partition_id_tensor else None
    in_names, out_names, out_avals = [], [], []
    for alloc in nc.m.functions[0].allocations:
        if not isinstance(alloc, mybir.MemoryLocationSet):
            continue
        name = alloc.memorylocations[0].name
        if alloc.kind == "ExternalInput":
            if name != partition_name:
                in_names.append(name)
        elif alloc.kind == "ExternalOutput":
            out_names.append(name)
            out_avals.append(
                jax.core.ShapedArray(tuple(alloc.tensor_shape), mybir.dt.np(alloc.dtype)))
    n_params = len(in_names)
    all_in = list(in_names) + list(out_names)

    def _pid():
        import jax.numpy as jnp
        from concourse.bass2jax import partition_id_tensor
        return partition_id_tensor()

    def _body(*args):
        operands = list(args)
        if partition_name is not None:
            operands.append(_pid())
        outs = _bass_exec_p.bind(
            *operands,
            out_avals=tuple(out_avals),
            in_names=tuple(all_in + ([partition_name] if partition_name else [])),
            out_names=tuple(out_names),
            lowering_input_output_aliases=(),
            sim_require_finite=True,
            sim_require_nnan=True,
            nc=nc,
        )
        return tuple(outs)

    devices = jax.devices()[:NCORES]
    mesh = Mesh(np.asarray(devices), ("core",))
    nin = n_params + len(out_names)
    sharded = jax.jit(shard_map(
        _body, mesh=mesh,
        in_specs=(PartitionSpec("core"),) * nin,
        out_specs=(PartitionSpec("core"),) * len(out_names),
        check_rep=False))

    def run(in_maps):
        concat_in = [
            np.concatenate([np.asarray(m[nm]) for m in in_maps], axis=0)
            for nm in in_names
        ]
        zeros = [np.zeros((NCORES * a.shape[0], *a.shape[1:]), a.dtype) for a in out_avals]
        out_arrs = sharded(*concat_in, *zeros)
        o = np.asarray(out_arrs[out_names.index("out")])
        return o.reshape(NCORES, T, D)

    runner = {"run": run, "sharded": sharded, "in_names": in_names,
              "out_names": out_names, "out_avals": out_avals}
    _CACHE["runner"] = runner
    return runner


def kernel(x, wq, bq, wk, bk, wv, bv, wo, bo, attn_mask):
    x = np.asarray(x, np.float32)
    in_maps = _make_in_maps(
        x, np.asarray(wq, np.float32), np.asarray(bq, np.float32),
        np.asarray(wk, np.float32), np.asarray(bk, np.float32),
        np.asarray(wv, np.float32), np.asarray(bv, np.float32),
        np.asarray(wo, np.float32))
    parts = _get_runner()["run"](in_maps)  # [8, T, D] (bf16 partials)
    parts = np.asarray(parts).astype(np.float32)
    out = parts.reshape(B, G, T, D).sum(axis=1) + np.asarray(bo, np.float32)
    return out.astype(np.float32)

